# revision 1
# baseline (speedup 1.0000x reference)
"""Causal self-attention (QK-RMSNorm + RoPE) Trainium2 kernel.

Sharding: 8 cores = 4 batches x 2 head-groups (Megatron-style over heads).
Core c handles batch b=c//2, heads [g*8, g*8+8) with g=c%2.
Each core computes y[b, :, g*512:(g+1)*512] (output-column sharding of the
projection after a pairwise AllGather of attention outputs), so the host
only concatenates slices - no host-side arithmetic.
"""


import numpy as np
import ml_dtypes

import concourse.bass as bass
import concourse.bacc as bacc

# Force all activations into the one table set that covers Exp+Ln+Square+
# Copy+Identity, so the whole kernel needs exactly one ACT_TABLE_LOAD.
import concourse.hw_specs as _hw_specs
_orig_gat = _hw_specs.get_activation_tables

def _gat_one_set(arch):
    t = _orig_gat(arch)
    return {k: (v if k == "natural_log_exp_and_others" else set())
            for k, v in t.items()}

bacc.get_activation_tables = _gat_one_set
import concourse.mybir as mybir
import concourse.tile as tile
from concourse.bass_utils import run_bass_kernel_spmd

BF16 = mybir.dt.bfloat16
F32 = mybir.dt.float32

N_HEAD = 16
HEAD_DIM = 64
EPS = 1e-5
ROPE_BASE = 10000.0

B, T, C = 4, 2048, 1024
H_LOCAL = N_HEAD // 2          # heads per core
PAIRS = H_LOCAL // 2           # head-pairs per core (processed 2-at-a-time)
CT = C // 128                  # contraction tiles over C
FL = H_LOCAL * HEAD_DIM        # local feature width (512)
QCH = 512                      # q-chunk width
NQC = T // QCH                 # q-chunks
NKT = T // 128                 # k tiles
NTT = T // 128                 # token tiles

_cached = {}


def _pbcast(ap, nparts):
    """Broadcast a [1, ...] AP across nparts partitions (partition step 0)."""
    return bass.AP(tensor=ap.tensor, offset=ap.offset, ap=[[0, nparts]] + ap.ap[1:])


def _fbcast2(ap):
    """[128, N] AP -> [128, 2, N] with the middle (free) dim broadcast."""
    return bass.AP(
        tensor=ap.tensor, offset=ap.offset, ap=[ap.ap[0], [0, 2], ap.ap[1]]
    )


def _rope_tables():
    inv_freq = 1.0 / (ROPE_BASE ** (np.arange(0, HEAD_DIM, 2, dtype=np.float64) / HEAD_DIM))
    t = np.arange(T, dtype=np.float64)
    freqs = np.outer(t, inv_freq)                       # [T, 32]
    emb = np.concatenate([freqs, freqs], -1)            # [T, 64]
    cos = np.cos(emb).astype(np.float32).T              # [64, T]
    sin = np.sin(emb).astype(np.float32).T              # [64, T]
    cos2 = np.concatenate([cos, cos], 0)                # [128, T] two heads
    sin_s = sin.copy()
    sin_s[0:32] = -sin_s[0:32]                          # rotate-half sign
    sin2 = np.concatenate([sin_s, sin_s], 0)            # [128, T]
    return cos2.astype(ml_dtypes.bfloat16), sin2.astype(ml_dtypes.bfloat16)


def _diag_masks():
    # corner mask: keep where k_partition <= q_col (lower-triangular 128x128)
    p = np.arange(128)[:, None]
    qf = np.arange(128)[None, :]
    m = (p <= qf).astype(np.float32)
    return m.astype(ml_dtypes.bfloat16)                 # [128, 128]


def build_program(no_cc=False):
    nc = bacc.Bacc("TRN2", target_bir_lowering=False, debug=False,
                   num_devices=1 if no_cc else 8)

    xT_d = nc.dram_tensor("xT", [C, T], BF16, kind="ExternalInput")
    wq_d = nc.dram_tensor("Wq", [C, FL], BF16, kind="ExternalInput")
    wk_d = nc.dram_tensor("Wk", [C, FL], BF16, kind="ExternalInput")
    wv_d = nc.dram_tensor("Wv", [C, FL], BF16, kind="ExternalInput")
    wp_d = nc.dram_tensor("Wp", [C, FL], BF16, kind="ExternalInput")
    y_d = nc.dram_tensor("y", [T, FL], F32, kind="ExternalOutput")

    cos2_np, sin2_np = _rope_tables()
    cos_d = nc.inline_tensor(np.ascontiguousarray(cos2_np), "cos2")
    sin_d = nc.inline_tensor(np.ascontiguousarray(sin2_np), "sin2")
    mask_d = nc.inline_tensor(np.ascontiguousarray(_diag_masks()), "masks")

    # per-pair exchange buffers
    cc_ins = [nc.dram_tensor(f"cc_in{p}", [128, T], BF16) for p in range(PAIRS)]
    cc_outs = [nc.dram_tensor(f"cc_out{p}", [2, 128, T], BF16) for p in range(PAIRS)]

    from contextlib import ExitStack
    with tile.TileContext(nc) as tc:
        with (
            tc.tile_pool(name="const", bufs=1) as const,
            tc.tile_pool(name="stats", bufs=8) as work,
            tc.tile_pool(name="evw", bufs=4) as evw,
            tc.tile_pool(name="rope", bufs=4) as ropep,
            tc.tile_pool(name="pt", bufs=5) as ptp,
            tc.tile_pool(name="ps_s2", bufs=2, space="PSUM") as ps_s2,
            tc.tile_pool(name="ps_yd", bufs=2, space="PSUM") as ps_yd,
            tc.tile_pool(name="ps_mm", bufs=2, space="PSUM") as ps_mm,
        ):
            early_ctx = ExitStack()
            early = early_ctx.enter_context(tc.tile_pool(name="early", bufs=1))

            # ---- constants / inputs ----
            xT_sb = early.tile([128, CT, T], BF16)
            nc.sync.dma_start(xT_sb[:], xT_d[:].rearrange("(k p) t -> p k t", p=128))
            wq_sb = early.tile([128, CT, FL], BF16)
            nc.sync.dma_start(wq_sb[:], wq_d[:].rearrange("(k p) f -> p k f", p=128))
            wk_sb = early.tile([128, CT, FL], BF16)
            nc.sync.dma_start(wk_sb[:], wk_d[:].rearrange("(k p) f -> p k f", p=128))
            wv_sb = early.tile([128, CT, FL], BF16)
            nc.sync.dma_start(wv_sb[:], wv_d[:].rearrange("(k p) f -> p k f", p=128))
            wp_sb = const.tile([128, CT, FL], BF16)
            nc.sync.dma_start(wp_sb[:], wp_d[:].rearrange("(k p) f -> p k f", p=128))
            cos_sb = early.tile([128, T], BF16)
            nc.sync.dma_start(cos_sb[:], cos_d[:])
            sin_sb = early.tile([128, T], BF16)
            nc.sync.dma_start(sin_sb[:], sin_d[:])
            mask_sb = early.tile([128, 128], BF16)
            nc.sync.dma_start(mask_sb[:], mask_d[:])
            ones_sb = const.tile([128, 64], BF16)
            nc.vector.memset(ones_sb[:], 1.0)

            qk_sb = const.tile([128, 2 * PAIRS, T], BF16)
            v_sb = const.tile([128, NTT, FL], BF16)
            aoT_sb = const.tile([128, PAIRS, T], BF16)

            def qkv_mtile(m):
                w_sb = wq_sb if m < PAIRS else wk_sb
                mloc = (m % PAIRS) * 128
                for n in range(T // QCH):
                    pss = ps_mm.tile([128, QCH], F32, tag="mm")
                    for k in range(CT):
                        nc.tensor.matmul(
                            pss,
                            lhsT=w_sb[:, k, mloc:mloc + 128],
                            rhs=xT_sb[:, k, n * QCH:(n + 1) * QCH],
                            start=(k == 0),
                            stop=(k == CT - 1),
                        )
                    sq = work.tile([128, QCH], BF16, tag="st")
                    nc.scalar.activation(sq[:], pss, mybir.ActivationFunctionType.Square)
                    ss = ps_mm.tile([128, QCH], F32, tag="mm")
                    nc.tensor.matmul(ss[0:64, :], lhsT=ones_sb[0:64, :], rhs=sq[0:64, :],
                                     start=True, stop=True, skip_group_check=True)
                    nc.tensor.matmul(ss[64:128, :], lhsT=ones_sb[64:128, :], rhs=sq[64:128, :],
                                     start=True, stop=True, skip_group_check=True)
                    # rstd = (ss/64)^(-1/2) = exp(-0.5*ln(ss/64)); eps is
                    # negligible vs mean-square ~1. ln+exp live in one ACT
                    # table set (natural_log_exp_and_others) -> no set thrash.
                    rr = work.tile([128, QCH], F32, tag="st")
                    nc.scalar.activation(rr[:], ss[:],
                                         mybir.ActivationFunctionType.Ln,
                                         scale=1.0 / HEAD_DIM)
                    rstd = work.tile([128, QCH], F32, tag="st")
                    nc.scalar.activation(rstd[:], rr[:],
                                         mybir.ActivationFunctionType.Exp,
                                         scale=-0.5)
                    dst = qk_sb[:, m, n * QCH:(n + 1) * QCH]
                    nc.vector.tensor_mul(dst, pss, rstd[:])

            def rope_mtile(m):
                src = qk_sb[:, m, :]
                sw = ropep.tile([128, T], BF16, tag="rp")
                for off in (0, 64):
                    nc.vector.tensor_copy(sw[off:off + 32, :], src[off + 32:off + 64, :])
                    nc.vector.tensor_copy(sw[off + 32:off + 64, :], src[off:off + 32, :])
                t1 = ropep.tile([128, T], BF16, tag="rp")
                nc.vector.tensor_mul(t1[:], src, cos_sb[:])
                nc.vector.tensor_mul(sw[:], sw[:], sin_sb[:])
                nc.vector.tensor_add(src, t1[:], sw[:])

            def attention_pair(p):
                qT = qk_sb[:, p, :]
                kT = qk_sb[:, PAIRS + p, :]
                for cqi in range(NQC):
                    kmax = (cqi + 1) * (QCH // 128)
                    yps = ps_yd.tile([128, QCH], F32, tag="yd")
                    dps = ps_yd.tile([128, QCH], F32, tag="yd")
                    for j in range(kmax):
                        jr = j - cqi * (QCH // 128)
                        q0 = max(jr, 0) * 128
                        sq_sl = slice(cqi * QCH + q0, (cqi + 1) * QCH)
                        s2 = ps_s2.tile([128, 2, QCH], F32, tag="s2")
                        nc.tensor.matmul(s2[:, 0, q0:QCH],
                                         lhsT=kT[0:64, j * 128:(j + 1) * 128],
                                         rhs=qT[0:64, sq_sl], start=True, stop=True)
                        nc.tensor.matmul(s2[:, 1, q0:QCH],
                                         lhsT=kT[64:128, j * 128:(j + 1) * 128],
                                         rhs=qT[64:128, sq_sl], start=True, stop=True)
                        pt = ptp.tile([128, 2, QCH], BF16, tag="pt")
                        nc.scalar.activation(pt[:, :, q0:QCH], s2[:, :, q0:QCH],
                                             mybir.ActivationFunctionType.Exp,
                                             scale=0.125)
                        if jr >= 0:
                            ptc = pt[:, :, q0:q0 + 128]
                            nc.vector.tensor_mul(ptc, ptc, _fbcast2(mask_sb[:]))
                        st, sp = (j == 0), (j == kmax - 1)
                        hA, hB = 2 * p, 2 * p + 1
                        nc.tensor.matmul(yps[0:64, q0:QCH],
                                         lhsT=v_sb[:, j, hA * 64:(hA + 1) * 64],
                                         rhs=pt[:, 0, q0:QCH], start=st, stop=sp,
                                         skip_group_check=True)
                        nc.tensor.matmul(yps[64:128, q0:QCH],
                                         lhsT=v_sb[:, j, hB * 64:(hB + 1) * 64],
                                         rhs=pt[:, 1, q0:QCH], start=st, stop=sp,
                                         skip_group_check=True)
                        nc.tensor.matmul(dps[0:64, q0:QCH], lhsT=ones_sb[:],
                                         rhs=pt[:, 0, q0:QCH], start=st, stop=sp,
                                         skip_group_check=True)
                        nc.tensor.matmul(dps[64:128, q0:QCH], lhsT=ones_sb[:],
                                         rhs=pt[:, 1, q0:QCH], start=st, stop=sp,
                                         skip_group_check=True)
                    dr = evw.tile([128, QCH], F32, tag="ev")
                    nc.vector.reciprocal_approx_fast(dr[:], dps[:])
                    dst = aoT_sb[:, p, cqi * QCH:(cqi + 1) * QCH]
                    nc.vector.tensor_mul(dst, yps[:], dr[:])

            # ---- v projection first (needed by every attention pair) ----
            for tt in range(NTT):
                pss = ps_mm.tile([128, FL], F32, tag="mm")
                for k in range(CT):
                    nc.tensor.matmul(
                        pss,
                        lhsT=xT_sb[:, k, tt * 128:(tt + 1) * 128],
                        rhs=wv_sb[:, k, :],
                        start=(k == 0),
                        stop=(k == CT - 1),
                    )
                nc.scalar.copy(v_sb[:, tt, :], pss)

            # ---- per-pair: qkv -> rope -> attention -> exchange ----
            for p in range(PAIRS):
                qkv_mtile(p)
                qkv_mtile(PAIRS + p)
                rope_mtile(p)
                rope_mtile(PAIRS + p)
                attention_pair(p)
                nc.sync.dma_start(cc_ins[p][:], aoT_sb[:, p, :])
                if not no_cc:
                    nc.gpsimd.collective_compute(
                        "AllGather",
                        mybir.AluOpType.bypass,
                        replica_groups=[[0, 1], [2, 3], [4, 5], [6, 7]],
                        ins=[cc_ins[p][:].opt()],
                        outs=[cc_outs[p][:].opt()],
                    )
                # qk slots p and 4+p are dead after attention p: receive the
                # gathered pair there (slot index == global f-tile index)
                if no_cc:
                    nc.sync.dma_start(qk_sb[:, p, :], cc_ins[p][:])
                    nc.sync.dma_start(qk_sb[:, PAIRS + p, :], cc_ins[p][:])
                else:
                    nc.sync.dma_start(qk_sb[:, p, :], cc_outs[p][0])
                    nc.sync.dma_start(qk_sb[:, PAIRS + p, :], cc_outs[p][1])

            early_ctx.close()

            # ---- projection over all 8 global f-tiles (rank-independent) ----
            for tt in range(NTT):
                pss = ps_mm.tile([128, FL], F32, tag="mm")
                # kf in exchange-arrival order (pair p delivers kf=p and kf=4+p)
                kf_order = [kf for p in range(PAIRS) for kf in (p, PAIRS + p)]
                for i, kf in enumerate(kf_order):
                    nc.tensor.matmul(
                        pss,
                        lhsT=qk_sb[:, kf, tt * 128:(tt + 1) * 128],
                        rhs=wp_sb[:, kf, :],
                        start=(i == 0),
                        stop=(i == 2 * PAIRS - 1),
                    )
                ysb = evw.tile([128, FL], F32, tag="ev")
                nc.scalar.copy(ysb[:], pss)
                nc.sync.dma_start(y_d[tt * 128:(tt + 1) * 128, :], ysb[:])

    nc.compile()
    return nc


def _prep_core_inputs(x, Wqkv, Wproj, q_norm_w, k_norm_w, core):
    b, g = core // 2, core % 2
    bf = ml_dtypes.bfloat16
    xT = np.ascontiguousarray(x[b].T).astype(bf)
    cols = slice(g * FL, (g + 1) * FL)
    wq = Wqkv[:, 0:C][:, cols] * np.tile(q_norm_w, H_LOCAL)[None, :]
    wk = Wqkv[:, C:2 * C][:, cols] * np.tile(k_norm_w, H_LOCAL)[None, :]
    wv = Wqkv[:, 2 * C:3 * C][:, cols]
    wp = Wproj[:, cols]
    return {
        "xT": xT,
        "Wq": np.ascontiguousarray(wq).astype(bf),
        "Wk": np.ascontiguousarray(wk).astype(bf),
        "Wv": np.ascontiguousarray(wv).astype(bf),
        "Wp": np.ascontiguousarray(wp).astype(bf),
    }


def kernel(x, Wqkv, Wproj, q_norm_w, k_norm_w):
    if "nc" not in _cached:
        _cached["nc"] = build_program()
    nc = _cached["nc"]

    x = np.asarray(x, dtype=np.float32)
    Wqkv = np.asarray(Wqkv, dtype=np.float32)
    Wproj = np.asarray(Wproj, dtype=np.float32)
    q_norm_w = np.asarray(q_norm_w, dtype=np.float32)
    k_norm_w = np.asarray(k_norm_w, dtype=np.float32)

    in_maps = [
        _prep_core_inputs(x, Wqkv, Wproj, q_norm_w, k_norm_w, c) for c in range(8)
    ]
    res = run_bass_kernel_spmd(nc, in_maps, list(range(8)))
    outs = res.results

    y = np.empty((B, T, C), dtype=np.float32)
    for b in range(B):
        y[b, :, 0:FL] = outs[2 * b]["y"]
        y[b, :, FL:C] = outs[2 * b + 1]["y"]
    return y



# revision 3
# speedup vs baseline: 1.1883x; 1.1883x over previous
"""Causal self-attention (QK-RMSNorm + RoPE) Trainium2 kernel, v2.

Sharding: 8 cores = 4 batches x 2 head-groups (Megatron-style over heads).
Core c handles batch b=c//2, heads [g*8, g*8+8) with g=c%2.
Each core computes y[b, :, g*512:(g+1)*512] (output-column sharding of the
projection after a pairwise AllGather of attention outputs), so the host
only concatenates slices - no host-side arithmetic.

v2 changes vs v1 (cost-model driven):
- AV matmuls restructured to [q-part, 65] outputs with a ones-column
  appended to V: the softmax denominator rides along for +1 PE cycle,
  eliminating the separate ones-matmul denominators and halving AV cost
  (PE attention work drops ~2x).
- Normalisation happens per q-tile on DVE (reciprocal_approx_fast +
  head-broadcast multiply); the [q, feat] result is transposed back to
  feat-major via DMA-engine xbar transposes (14ns/tile, no PE/Act cost).
- Input DMAs are chunked along T so the V projection starts ~4us in
  instead of waiting ~23us for whole-tensor loads.
- PSUM drains for V/proj moved from Act to DVE (Act is exp-bound).
- RoPE swap-copies replaced by partition-offset multiplies (6 DVE ops
  per m-tile instead of 7).
- Next pair's QKV chunks are interleaved into the (Act-bound) attention
  phase so the PE stays busy while exp runs.
"""


import numpy as np
import ml_dtypes

import concourse.bass as bass
import concourse.bacc as bacc

# Force all activations into the one table set that covers Exp+Ln+Square+
# Copy+Identity, so the whole kernel needs exactly one ACT_TABLE_LOAD.
import concourse.hw_specs as _hw_specs
_orig_gat = _hw_specs.get_activation_tables

def _gat_one_set(arch):
    t = _orig_gat(arch)
    return {k: (v if k == "natural_log_exp_and_others" else set())
            for k, v in t.items()}

bacc.get_activation_tables = _gat_one_set
import concourse.mybir as mybir
import concourse.tile as tile
from concourse.bass_utils import run_bass_kernel_spmd

BF16 = mybir.dt.bfloat16
F32 = mybir.dt.float32

N_HEAD = 16
HEAD_DIM = 64
EPS = 1e-5
ROPE_BASE = 10000.0

B, T, C = 4, 2048, 1024
H_LOCAL = N_HEAD // 2          # heads per core
PAIRS = H_LOCAL // 2           # head-pairs per core (processed 2-at-a-time)
CT = C // 128                  # contraction tiles over C
FL = H_LOCAL * HEAD_DIM        # local feature width (512)
QCH = 512                      # q-chunk width
NQC = T // QCH                 # q-chunks
NKT = T // 128                 # k tiles
NTT = T // 128                 # token tiles
VW = HEAD_DIM + 1              # V tile width incl. ones column (65)

_cached = {}


def _pbcast(ap, nparts):
    """Broadcast a [1, ...] AP across nparts partitions (partition step 0)."""
    return bass.AP(tensor=ap.tensor, offset=ap.offset, ap=[[0, nparts]] + ap.ap[1:])


def _fbcast2(ap):
    """[128, N] AP -> [128, 2, N] with the middle (free) dim broadcast."""
    return bass.AP(
        tensor=ap.tensor, offset=ap.offset, ap=[ap.ap[0], [0, 2], ap.ap[1]]
    )


def _hbcast(ap, n):
    """[128, 2] AP -> [128, 2, n] with the last (free) dim broadcast."""
    return bass.AP(
        tensor=ap.tensor, offset=ap.offset, ap=[ap.ap[0], ap.ap[1], [0, n]]
    )


def _rope_tables():
    inv_freq = 1.0 / (ROPE_BASE ** (np.arange(0, HEAD_DIM, 2, dtype=np.float64) / HEAD_DIM))
    t = np.arange(T, dtype=np.float64)
    freqs = np.outer(t, inv_freq)                       # [T, 32]
    emb = np.concatenate([freqs, freqs], -1)            # [T, 64]
    cos = np.cos(emb).astype(np.float32).T              # [64, T]
    sin = np.sin(emb).astype(np.float32).T              # [64, T]
    cos2 = np.concatenate([cos, cos], 0)                # [128, T] two heads
    sin_s = sin.copy()
    sin_s[0:32] = -sin_s[0:32]                          # rotate-half sign
    sin2 = np.concatenate([sin_s, sin_s], 0)            # [128, T]
    return cos2.astype(ml_dtypes.bfloat16), sin2.astype(ml_dtypes.bfloat16)


def _diag_masks():
    # corner mask: keep where k_partition <= q_col (lower-triangular 128x128)
    p = np.arange(128)[:, None]
    qf = np.arange(128)[None, :]
    m = (p <= qf).astype(np.float32)
    return m.astype(ml_dtypes.bfloat16)                 # [128, 128]


def build_program(no_cc=False):
    nc = bacc.Bacc("TRN2", target_bir_lowering=False, debug=False,
                   num_devices=1 if no_cc else 8)

    xT_d = nc.dram_tensor("xT", [C, T], BF16, kind="ExternalInput")
    wq_d = nc.dram_tensor("Wq", [C, FL], BF16, kind="ExternalInput")
    wk_d = nc.dram_tensor("Wk", [C, FL], BF16, kind="ExternalInput")
    wv_d = nc.dram_tensor("Wv", [C, FL], BF16, kind="ExternalInput")
    wp_d = nc.dram_tensor("Wp", [C, FL], BF16, kind="ExternalInput")
    y_d = nc.dram_tensor("y", [T, FL], F32, kind="ExternalOutput")

    cos2_np, sin2_np = _rope_tables()
    cos_d = nc.inline_tensor(np.ascontiguousarray(cos2_np), "cos2")
    sin_d = nc.inline_tensor(np.ascontiguousarray(sin2_np), "sin2")
    mask_d = nc.inline_tensor(np.ascontiguousarray(_diag_masks()), "masks")

    # per-pair exchange buffers
    cc_ins = [nc.dram_tensor(f"cc_in{p}", [128, T], BF16) for p in range(PAIRS)]
    cc_outs = [nc.dram_tensor(f"cc_out{p}", [2, 128, T], BF16) for p in range(PAIRS)]

    from contextlib import ExitStack
    with tile.TileContext(nc) as tc:
        with (
            tc.tile_pool(name="const", bufs=1) as const,
            tc.tile_pool(name="stats", bufs=8) as work,
            tc.tile_pool(name="evw", bufs=8) as evw,
            tc.tile_pool(name="rope", bufs=4) as ropep,
            tc.tile_pool(name="pt", bufs=6) as ptp,
            tc.tile_pool(name="aop", bufs=2) as aop,
            tc.tile_pool(name="ps_s2", bufs=2, space="PSUM") as ps_s2,
            tc.tile_pool(name="ps_av", bufs=1, space="PSUM") as ps_av,
            tc.tile_pool(name="ps_mm", bufs=2, space="PSUM") as ps_mm,
        ):
            early_ctx = ExitStack()
            early = early_ctx.enter_context(tc.tile_pool(name="early", bufs=1))

            # ---- constants / inputs (chunked along T so compute starts early)
            wv_sb = early.tile([128, CT, FL], BF16)
            nc.sync.dma_start(wv_sb[:], wv_d[:].rearrange("(k p) f -> p k f", p=128))
            xT_sb = early.tile([128, CT, T], BF16)
            xr = xT_d[:].rearrange("(k p) t -> p k t", p=128)
            for tch in range(NQC):
                sl = slice(tch * QCH, (tch + 1) * QCH)
                nc.sync.dma_start(xT_sb[:, :, sl], xr[:, :, sl])
            wq_sb = early.tile([128, CT, FL], BF16)
            nc.sync.dma_start(wq_sb[:], wq_d[:].rearrange("(k p) f -> p k f", p=128))
            wk_sb = early.tile([128, CT, FL], BF16)
            nc.sync.dma_start(wk_sb[:], wk_d[:].rearrange("(k p) f -> p k f", p=128))
            cos_sb = early.tile([128, T], BF16)
            nc.sync.dma_start(cos_sb[:], cos_d[:])
            sin_sb = early.tile([128, T], BF16)
            nc.sync.dma_start(sin_sb[:], sin_d[:])
            mask_sb = early.tile([128, 128], BF16)
            nc.sync.dma_start(mask_sb[:], mask_d[:])
            wp_sb = const.tile([128, CT, FL], BF16)
            nc.sync.dma_start(wp_sb[:], wp_d[:].rearrange("(k p) f -> p k f", p=128))
            ones_sb = const.tile([128, 64], BF16)
            nc.vector.memset(ones_sb[:], 1.0)

            qk_sb = const.tile([128, 2 * PAIRS, T], BF16)
            # V extended: [k-tile, head, 64 vals + ones col]
            v_sb = const.tile([128, NTT, H_LOCAL, VW], BF16)
            nc.vector.memset(v_sb[:, :, :, HEAD_DIM:VW], 1.0)
            aoT_sb = const.tile([128, PAIRS, T], BF16)

            def qkv_unit(m, n):
                """One 512-col chunk of the q/k projection for m-tile m."""
                w_sb = wq_sb if m < PAIRS else wk_sb
                mloc = (m % PAIRS) * 128
                pss = ps_mm.tile([128, QCH], F32, tag="mm")
                for k in range(CT):
                    nc.tensor.matmul(
                        pss,
                        lhsT=w_sb[:, k, mloc:mloc + 128],
                        rhs=xT_sb[:, k, n * QCH:(n + 1) * QCH],
                        start=(k == 0),
                        stop=(k == CT - 1),
                    )
                sq = work.tile([128, QCH], BF16, tag="st")
                nc.scalar.activation(sq[:], pss, mybir.ActivationFunctionType.Square)
                ss = ps_mm.tile([128, QCH], F32, tag="mm")
                nc.tensor.matmul(ss[0:64, :], lhsT=ones_sb[0:64, :], rhs=sq[0:64, :],
                                 start=True, stop=True, skip_group_check=True)
                nc.tensor.matmul(ss[64:128, :], lhsT=ones_sb[64:128, :], rhs=sq[64:128, :],
                                 start=True, stop=True, skip_group_check=True)
                # rstd = (ss/64)^(-1/2) = exp(-0.5*ln(ss/64)); eps is
                # negligible vs mean-square ~1. ln+exp live in one ACT
                # table set (natural_log_exp_and_others) -> no set thrash.
                rr = work.tile([128, QCH], F32, tag="st")
                nc.scalar.activation(rr[:], ss[:],
                                     mybir.ActivationFunctionType.Ln,
                                     scale=1.0 / HEAD_DIM)
                rstd = work.tile([128, QCH], F32, tag="st")
                nc.scalar.activation(rstd[:], rr[:],
                                     mybir.ActivationFunctionType.Exp,
                                     scale=-0.5)
                dst = qk_sb[:, m, n * QCH:(n + 1) * QCH]
                nc.vector.tensor_mul(dst, pss, rstd[:])

            def rope_mtile(m):
                """q' = q*cos + rot(q)*sin via partition-offset muls (no copies)."""
                src = qk_sb[:, m, :]
                t1 = ropep.tile([128, T], BF16, tag="rp")
                nc.vector.tensor_mul(t1[:], src, cos_sb[:])
                sw = ropep.tile([128, T], BF16, tag="rp")
                for off in (0, 64):
                    nc.vector.tensor_mul(sw[off:off + 32, :],
                                         src[off + 32:off + 64, :],
                                         sin_sb[off:off + 32, :])
                    nc.vector.tensor_mul(sw[off + 32:off + 64, :],
                                         src[off:off + 32, :],
                                         sin_sb[off + 32:off + 64, :])
                nc.vector.tensor_add(src, t1[:], sw[:])

            def attention_pair(p, filler=None):
                """Attention for head-pair p. AV outputs [q-part, 65] per head
                (ones col = softmax denominator). filler() emits next-pair QKV
                chunks to keep the PE busy while Act runs exp."""
                qT = qk_sb[:, p, :]
                kT = qk_sb[:, PAIRS + p, :]
                ao = aop.tile([128, NTT, 128], BF16, tag="ao")
                for cqi in range(NQC):
                    kmax = (cqi + 1) * (QCH // 128)
                    yav = ps_av.tile([128, 4, 2, VW], F32, tag="av")
                    pending = []

                    def emit_avd(j, pt):
                        jr = j - cqi * (QCH // 128)
                        for qt in range(max(jr, 0), 4):
                            for h in (0, 1):
                                nc.tensor.matmul(
                                    yav[:, qt, h, :],
                                    lhsT=pt[:, h, qt * 128:(qt + 1) * 128],
                                    rhs=v_sb[:, j, 2 * p + h, :],
                                    start=(j == 0),
                                    stop=(j == cqi * 4 + qt),
                                    skip_group_check=True,
                                )

                    for j in range(kmax):
                        jr = j - cqi * (QCH // 128)
                        q0 = max(jr, 0) * 128
                        sq_sl = slice(cqi * QCH + q0, (cqi + 1) * QCH)
                        s2 = ps_s2.tile([128, 2, QCH], F32, tag="s2")
                        nc.tensor.matmul(s2[:, 0, q0:QCH],
                                         lhsT=kT[0:64, j * 128:(j + 1) * 128],
                                         rhs=qT[0:64, sq_sl], start=True, stop=True)
                        nc.tensor.matmul(s2[:, 1, q0:QCH],
                                         lhsT=kT[64:128, j * 128:(j + 1) * 128],
                                         rhs=qT[64:128, sq_sl], start=True, stop=True)
                        pt = ptp.tile([128, 2, QCH], BF16, tag="pt")
                        nc.scalar.activation(pt[:, :, q0:QCH], s2[:, :, q0:QCH],
                                             mybir.ActivationFunctionType.Exp,
                                             scale=0.125)
                        if jr >= 0:
                            ptc = pt[:, :, q0:q0 + 128]
                            nc.vector.tensor_mul(ptc, ptc, _fbcast2(mask_sb[:]))
                        pending.append((j, pt))
                        if len(pending) > 3:
                            emit_avd(*pending.pop(0))
                    for ent in pending:
                        emit_avd(*ent)

                    # normalize per q-tile on DVE, then xbar-transpose to
                    # feat-major (runs on the DMA engines, not PE/Act).
                    for qt in range(4):
                        qa = cqi * 4 + qt
                        dr = evw.tile([128, 2], F32, tag="dr")
                        nc.vector.reciprocal_approx_fast(
                            dr[:], yav[:, qt, :, HEAD_DIM:VW])
                        nc.vector.tensor_mul(
                            ao[:, qa, :].rearrange("p (h d) -> p h d", h=2),
                            yav[:, qt, :, 0:HEAD_DIM],
                            _hbcast(dr[:], HEAD_DIM))
                        nc.sync.dma_start(
                            aoT_sb[:, p, qa * 128:(qa + 1) * 128],
                            ao[:, qa, :], transpose=True)
                    if filler is not None:
                        filler(cqi)

            # ---- v projection first (needed by attention pair 0) ----
            for tt in range(NTT):
                pss = ps_mm.tile([128, FL], F32, tag="mm")
                for k in range(CT):
                    nc.tensor.matmul(
                        pss,
                        lhsT=xT_sb[:, k, tt * 128:(tt + 1) * 128],
                        rhs=wv_sb[:, k, :],
                        start=(k == 0),
                        stop=(k == CT - 1),
                    )
                nc.vector.tensor_copy(
                    v_sb[:, tt, :, 0:HEAD_DIM],
                    pss.rearrange("p (h d) -> p h d", h=H_LOCAL))

            # ---- pair 0 qkv + rope up front ----
            for m in (0, PAIRS):
                for n in range(NQC):
                    qkv_unit(m, n)
            rope_mtile(0)
            rope_mtile(PAIRS)

            # ---- per-pair: attention (with next-pair qkv interleaved) ----
            for p in range(PAIRS):
                if p + 1 < PAIRS:
                    units = [(m, n) for m in (p + 1, PAIRS + p + 1)
                             for n in range(NQC)]

                    def filler(cqi, units=units):
                        for m, n in units[cqi * 2:(cqi + 1) * 2]:
                            qkv_unit(m, n)
                else:
                    filler = None
                attention_pair(p, filler)
                if p + 1 < PAIRS:
                    rope_mtile(p + 1)
                    rope_mtile(PAIRS + p + 1)
                nc.sync.dma_start(cc_ins[p][:], aoT_sb[:, p, :])
                if not no_cc:
                    nc.gpsimd.collective_compute(
                        "AllGather",
                        mybir.AluOpType.bypass,
                        replica_groups=[[0, 1], [2, 3], [4, 5], [6, 7]],
                        ins=[cc_ins[p][:].opt()],
                        outs=[cc_outs[p][:].opt()],
                    )
                # qk slots p and 4+p are dead after attention p: receive the
                # gathered pair there (slot index == global f-tile index)
                if no_cc:
                    nc.sync.dma_start(qk_sb[:, p, :], cc_ins[p][:])
                    nc.sync.dma_start(qk_sb[:, PAIRS + p, :], cc_ins[p][:])
                else:
                    nc.sync.dma_start(qk_sb[:, p, :], cc_outs[p][0])
                    nc.sync.dma_start(qk_sb[:, PAIRS + p, :], cc_outs[p][1])

            early_ctx.close()

            # ---- projection over all 8 global f-tiles (rank-independent) ----
            for tt in range(NTT):
                pss = ps_mm.tile([128, FL], F32, tag="mm")
                # kf in exchange-arrival order (pair p delivers kf=p and kf=4+p)
                kf_order = [kf for p in range(PAIRS) for kf in (p, PAIRS + p)]
                for i, kf in enumerate(kf_order):
                    nc.tensor.matmul(
                        pss,
                        lhsT=qk_sb[:, kf, tt * 128:(tt + 1) * 128],
                        rhs=wp_sb[:, kf, :],
                        start=(i == 0),
                        stop=(i == 2 * PAIRS - 1),
                    )
                ysb = evw.tile([128, FL], F32, tag="ev")
                nc.vector.tensor_copy(ysb[:], pss)
                nc.sync.dma_start(y_d[tt * 128:(tt + 1) * 128, :], ysb[:])

    nc.compile()
    return nc


def _prep_core_inputs(x, Wqkv, Wproj, q_norm_w, k_norm_w, core):
    b, g = core // 2, core % 2
    bf = ml_dtypes.bfloat16
    xT = np.ascontiguousarray(x[b].T).astype(bf)
    cols = slice(g * FL, (g + 1) * FL)
    wq = Wqkv[:, 0:C][:, cols] * np.tile(q_norm_w, H_LOCAL)[None, :]
    wk = Wqkv[:, C:2 * C][:, cols] * np.tile(k_norm_w, H_LOCAL)[None, :]
    wv = Wqkv[:, 2 * C:3 * C][:, cols]
    wp = Wproj[:, cols]
    return {
        "xT": xT,
        "Wq": np.ascontiguousarray(wq).astype(bf),
        "Wk": np.ascontiguousarray(wk).astype(bf),
        "Wv": np.ascontiguousarray(wv).astype(bf),
        "Wp": np.ascontiguousarray(wp).astype(bf),
    }


def kernel(x, Wqkv, Wproj, q_norm_w, k_norm_w):
    if "nc" not in _cached:
        _cached["nc"] = build_program()
    nc = _cached["nc"]

    x = np.asarray(x, dtype=np.float32)
    Wqkv = np.asarray(Wqkv, dtype=np.float32)
    Wproj = np.asarray(Wproj, dtype=np.float32)
    q_norm_w = np.asarray(q_norm_w, dtype=np.float32)
    k_norm_w = np.asarray(k_norm_w, dtype=np.float32)

    in_maps = [
        _prep_core_inputs(x, Wqkv, Wproj, q_norm_w, k_norm_w, c) for c in range(8)
    ]
    res = run_bass_kernel_spmd(nc, in_maps, list(range(8)))
    outs = res.results

    y = np.empty((B, T, C), dtype=np.float32)
    for b in range(B):
        y[b, :, 0:FL] = outs[2 * b]["y"]
        y[b, :, FL:C] = outs[2 * b + 1]["y"]
    return y
